# revision 1
# baseline (speedup 1.0000x reference)
"""Top-1 MoE routing layer (HCE Linear) on 8 Trainium2 NeuronCores.

y[b] = x[b] @ W[argmax_e sigmoid(x @ Wp.T + bp)[b, e]]   (multi-hot on exact ties)

Strategy: data-parallel over tokens. The router (a [B,8] matmul + argmax —
~0.03% of the FLOPs) is computed on host in fp32 with exactly the reference
semantics; tokens are then grouped by expert into capacity-padded segments so
all 8 cores run one identical (SPMD) Bass program that is just dense
per-segment fp32 matmuls at the HBM roofline (~3 MB/core: 2 MB weights +
~0.5 MB activations each way).
"""

from contextlib import ExitStack

import numpy as np

import bass_rust
import concourse.bass as bass
import concourse.tile as tile
from concourse import mybir
from concourse.bass_utils import run_bass_kernel_spmd
from concourse.vector_clock import ScopedClock

NCORES = 8


class _SplitDrainTileContext(tile.TileContext):
    """TileContext legalized for a walrus build that allows at most ONE sem
    wait per instruction ("Too many sync wait commands" otherwise).

    Extra waits are hoisted onto same-engine InstNoOp carriers placed
    immediately before the owning instruction (identical semantics: the
    engine sequencer executes them in order), and the kernel-tail drain is
    split into a chain of single-wait drains.
    """

    _wait_nop_counter = 0

    def _lower_ordered_insts(self, ordered):
        for bb_name, insts in list(ordered.items()):
            out = []
            for inst in insts:
                si = getattr(inst, "sync_info", None)
                waits = list(si.on_wait) if si is not None else []
                if len(waits) > 1:
                    for w in waits[:-1]:
                        type(self)._wait_nop_counter += 1
                        nop = mybir.InstNoOp(
                            name=f"waitnop_{type(self)._wait_nop_counter}",
                            engine=inst.engine,
                            sync_info=mybir.SyncInfo(on_wait=[w], on_update=[]),
                            bass_nofuse=True,
                        )
                        out.append(nop)
                    inst.sync_info = mybir.SyncInfo(
                        on_wait=[waits[-1]], on_update=list(si.on_update)
                    )
                out.append(inst)
            ordered[bb_name] = out
        return super()._lower_ordered_insts(ordered)

    def _drain_and_barrier(self, tick_clock, wait_clock):
        drain_inst = self.nc.sync.drain()
        wait_clock.add_sem_waits(
            drain_inst.ins, ScopedClock({None: tick_clock.global_clock})
        )
        si = drain_inst.ins.sync_info
        waits = list(si.on_wait)
        if len(waits) > 1:
            # strip the drain; carry each wait on a cheap nop instead of a
            # chain of full drains (those cost ~100ns each)
            drain_inst.ins.sync_info = bass_rust.SyncInfo(
                on_wait=[], on_update=list(si.on_update)
            )
            for w in waits:
                n2 = self.nc.sync.nop(nofuse=True)
                n2.ins.sync_info = bass_rust.SyncInfo(on_wait=[w], on_update=[])
        self.nc.all_engine_barrier(sem_only=True)
        assert self.sems is not None
        popped = self.nc._tile_sem_poison_stack.pop()
        assert popped is self._sem_poison
        self.nc.clear_and_free_semaphores(list(self.sems.allocated().values()))


def _build_program(I, O, E, C):
    """One SPMD core program: yT[o, seg] = W[e(seg)].T @ xT[:, seg].

    Inputs (per core):
      wk [2, 128, E*2*128] fp32 — wk[ot, p, (e*2+kt)*128 + c] = W[e, kt*128+p, ot*128+c]
      xk [128, 2, E*C]     fp32 — xk[p, kt, e*C+j] = x_token(e,j)[kt*128+p]
    Output:
      yk [2, 128, E*C]     fp32 — yk[ot, p, col] = y_col[ot*128+p]
    """
    assert I == 256 and O == 256, "packed layout assumes 256x256 experts"
    KT = I // 128  # 2
    OT = O // 128  # 2
    S = E * C  # total columns per core
    dt = mybir.dt.float32

    nc = bass.Bass("TRN2", target_bir_lowering=False, debug=False, num_devices=NCORES)
    wk = nc.dram_tensor("wk", [OT, 128, E * KT * 128], dt, kind="ExternalInput").ap()
    xk = nc.dram_tensor("xk", [128, KT, S], dt, kind="ExternalInput").ap()
    yk = nc.dram_tensor("yk", [OT, 128, S], dt, kind="ExternalOutput").ap()

    # segment -> bank-aligned matmul pieces (PSUM bank = 512 fp32 columns)
    pieces = []  # (e, col_start, n)
    for e in range(E):
        s0, s1 = e * C, (e + 1) * C
        while s0 < s1:
            nxt = min(s1, (s0 // 512 + 1) * 512)
            pieces.append((e, s0, nxt - s0))
            s0 = nxt

    with _SplitDrainTileContext(nc) as tc:
        with ExitStack() as ctx:
            wpool = ctx.enter_context(tc.tile_pool(name="w", bufs=OT))
            xpool = ctx.enter_context(tc.tile_pool(name="x", bufs=1))
            ppool = ctx.enter_context(tc.tile_pool(name="ps", bufs=OT, space="PSUM"))
            ypool = ctx.enter_context(tc.tile_pool(name="y", bufs=OT))

            # two HWDGE rings (SP + ACT). x halves land first (they gate all
            # matmuls); W arrives in tapered chunks per ot so the last chunk
            # gates as little work as possible.
            XH = S // 2
            sbx = []
            for h, eng in ((0, nc.scalar), (1, nc.sync)):
                t = xpool.tile([128, KT, XH], dt, tag=f"x{h}")
                sbx.append((t, eng))

            def rhs_ap(kt, s0, n):
                h = s0 // XH
                assert (s0 + n - 1) // XH == h, (s0, n)
                return sbx[h][0][:, kt, s0 - h * XH : s0 - h * XH + n]

            # tapered expert chunks; big chunks first on each ring, small last
            CH = [(0, 4), (4, 3), (7, 1)]  # (first expert, n experts)
            sbw = {}

            def issue_w(ot, ci, eng):
                e0, ne = CH[ci]
                t = wpool.tile([128, ne * KT * 128], dt, tag=f"w{ot}{ci}")
                lo = e0 * KT * 128
                eng.dma_start(out=t[:], in_=wk[ot, :, lo : lo + ne * KT * 128])
                sbw[(ot, ci)] = t

            def issue_x(h):
                t, eng = sbx[h]
                eng.dma_start(out=t[:], in_=xk[:, :, h * XH : (h + 1) * XH])

            issue_x(0)
            issue_x(1)
            issue_w(1, 0, nc.sync)
            issue_w(0, 0, nc.scalar)
            issue_w(0, 1, nc.sync)
            issue_w(1, 1, nc.scalar)
            issue_w(0, 2, nc.sync)
            issue_w(1, 2, nc.scalar)

            ps = []
            sby = []
            for ot in range(OT):
                ps_t = ppool.tile([128, S], dt, tag=f"ps{ot}")
                ps.append(ps_t)
                sby_t = ypool.tile([128, S], dt, tag=f"sy{ot}")
                sby.append(sby_t)
            for ci, (e0, ne) in enumerate(CH):
                ot_order = [1, 0] if ci == 0 else list(range(OT))
                for ot in ot_order:
                    for e, s0, n in pieces:
                        if not (e0 <= e < e0 + ne):
                            continue
                        for kt in range(KT):
                            te = ((e - e0) * KT + kt) * 128
                            nc.tensor.matmul(
                                out=ps[ot][:, s0 : s0 + n],
                                lhsT=sbw[(ot, ci)][:, te : te + 128],
                                rhs=rhs_ap(kt, s0, n),
                                start=(kt == 0),
                                stop=(kt == KT - 1),
                            )
                for ot in ot_order:
                    lo, hi = e0 * C, (e0 + ne) * C
                    nc.vector.tensor_copy(sby[ot][:, lo:hi], ps[ot][:, lo:hi])
                    eng = nc.scalar if ot == 0 else nc.sync
                    eng.dma_start(out=yk[ot, :, lo:hi], in_=sby[ot][:, lo:hi])

    return nc


_cache: dict = {}


def _get_program(I, O, E, C):
    key = (I, O, E, C)
    if key not in _cache:
        _cache[key] = _build_program(I, O, E, C)
    return _cache[key]


def _pack_inputs(x, W, Wp, bp):
    B, I = x.shape
    E, _, O = W.shape

    # --- host router: replicate reference fp32 semantics (incl. tie multi-hot)
    logits = x @ Wp.T + bp
    g = 1.0 / (1.0 + np.exp(-logits, dtype=np.float32))
    onehot = g == g.max(axis=1, keepdims=True)  # [B, E] bool, >=1 True per row
    tok_of_pair, exp_of_pair = np.nonzero(onehot)  # pairs sorted by token

    # per-expert pair lists, split evenly over cores into capacity-C segments
    order = np.argsort(exp_of_pair, kind="stable")
    toks_by_e = tok_of_pair[order]
    n_e = np.bincount(exp_of_pair, minlength=E)
    C = max(1, int(-(-n_e.max() // NCORES)))  # ceil(max_e n_e / NCORES)
    S = E * C

    # slot tables: for each (core, e, j<cnt) the source token
    src_tok = np.zeros((NCORES, S), dtype=np.int64)
    valid = np.zeros((NCORES, S), dtype=bool)
    off = 0
    for e in range(E):
        parts = np.array_split(toks_by_e[off : off + n_e[e]], NCORES)
        off += n_e[e]
        for c in range(NCORES):
            k = len(parts[c])
            src_tok[c, e * C : e * C + k] = parts[c]
            valid[c, e * C : e * C + k] = True

    # pack inputs
    wkk = (
        W.reshape(E, 2, 128, 2, 128).transpose(3, 2, 0, 1, 4).reshape(2, 128, E * 2 * 128)
    )
    wkk = np.ascontiguousarray(wkk)
    xT = np.ascontiguousarray(x.T.reshape(2, 128, B))  # [kt, p, b]
    in_maps = []
    for c in range(NCORES):
        xs = np.zeros((128, 2, S), dtype=np.float32)
        cols = np.nonzero(valid[c])[0]
        xs[:, :, cols] = xT.transpose(1, 0, 2)[:, :, src_tok[c, cols]]
        in_maps.append({"wk": wkk, "xk": xs})
    return in_maps, (C, S, src_tok, valid)


def kernel(x, W, Wp, bp):
    x = np.ascontiguousarray(np.asarray(x, dtype=np.float32))
    W = np.ascontiguousarray(np.asarray(W, dtype=np.float32))
    Wp = np.ascontiguousarray(np.asarray(Wp, dtype=np.float32))
    bp = np.ascontiguousarray(np.asarray(bp, dtype=np.float32))
    B, I = x.shape
    E, _, O = W.shape

    in_maps, (C, S, src_tok, valid) = _pack_inputs(x, W, Wp, bp)
    nc = _get_program(I, O, E, C)
    res = run_bass_kernel_spmd(nc, in_maps, list(range(NCORES)))

    # host unscatter: y[token] += yT column (add: handles tie multi-hot rows)
    y = np.zeros((B, O), dtype=np.float32)
    for c in range(NCORES):
        yc = res.results[c]["yk"]  # [2, 128, S]
        ycol = yc.transpose(2, 0, 1).reshape(S, O)  # [S, O]
        cols = np.nonzero(valid[c])[0]
        np.add.at(y, src_tok[c, cols], ycol[cols])
    return y



# revision 7
# speedup vs baseline: 1.4332x; 1.4332x over previous
"""Top-1 MoE routing layer (HCE Linear) on 8 Trainium2 NeuronCores.

y[b] = x[b] @ W[argmax_e sigmoid(x @ Wp.T + bp)[b, e]]   (multi-hot on exact ties)

Strategy: EXPERT-parallel. The router runs on host in fp32 with exactly the
reference semantics; core e receives only expert e's weight (fp16) and the
tokens routed to it, packed x^T-style so the PE produces y token-major.
Inputs arrive as one HBM blob in a few HWDGE chunks; outputs leave through
SWDGE scatter-adds whose descriptors are PREPARED during the input-DMA dead
time (compile-time iota indices) and merely triggered when each y piece is
ready — skipping the HWDGE hold + DGE delay on the latency-critical tail.
"""

from contextlib import ExitStack

import numpy as np

import bass_rust
import concourse.bass as bass
import concourse.tile as tile
from concourse import library_config, mybir
from concourse.bass_utils import run_bass_kernel_spmd
from concourse.vector_clock import ScopedClock

NCORES = 8
E = 8
I = 256
O = 256


class _SplitDrainTileContext(tile.TileContext):
    """TileContext legalized for a walrus build that allows at most ONE sem
    wait per instruction ("Too many sync wait commands" otherwise).

    Extra waits are hoisted onto same-engine InstNoOp carriers placed
    immediately before the owning instruction (identical semantics: the
    engine sequencer executes them in order), and the kernel-tail drain is
    split into a chain of single-wait drains.
    """

    _wait_nop_counter = 0

    def _lower_ordered_insts(self, ordered):
        for bb_name, insts in list(ordered.items()):
            out = []
            for inst in insts:
                si = getattr(inst, "sync_info", None)
                waits = list(si.on_wait) if si is not None else []
                if len(waits) > 1:
                    for w in waits[:-1]:
                        type(self)._wait_nop_counter += 1
                        nop = mybir.InstNoOp(
                            name=f"waitnop_{type(self)._wait_nop_counter}",
                            engine=inst.engine,
                            sync_info=mybir.SyncInfo(on_wait=[w], on_update=[]),
                            bass_nofuse=True,
                        )
                        out.append(nop)
                    inst.sync_info = mybir.SyncInfo(
                        on_wait=[waits[-1]], on_update=list(si.on_update)
                    )
                out.append(inst)
            ordered[bb_name] = out
        return super()._lower_ordered_insts(ordered)

    def _drain_and_barrier(self, tick_clock, wait_clock):
        drain_inst = self.nc.sync.drain()
        wait_clock.add_sem_waits(
            drain_inst.ins, ScopedClock({None: tick_clock.global_clock})
        )
        si = drain_inst.ins.sync_info
        waits = list(si.on_wait)
        if len(waits) > 1:
            # strip the drain; carry each wait on a cheap nop instead of a
            # chain of full drains (those cost ~100ns each)
            drain_inst.ins.sync_info = bass_rust.SyncInfo(
                on_wait=[], on_update=list(si.on_update)
            )
            for w in waits:
                n2 = self.nc.sync.nop(nofuse=True)
                n2.ins.sync_info = bass_rust.SyncInfo(on_wait=[w], on_update=[])
        self.nc.all_engine_barrier(sem_only=True)
        assert self.sems is not None
        popped = self.nc._tile_sem_poison_stack.pop()
        assert popped is self._sem_poison
        self.nc.clear_and_free_semaphores(list(self.sems.allocated().values()))


def _plan(ntiles):
    """Static schedule parameters for a given tile count.

    chunk_tiles: tiles per input DMA chunk (chunk 0 also carries W).
    piece_tiles: tiles per output scatter piece.
    """
    if ntiles <= 2:
        chunks = [ntiles]
    elif ntiles == 3:
        chunks = [2, 1]
    else:
        a = 2
        rest = ntiles - a
        b = (rest + 1) // 2
        chunks = [a, b, rest - b] if rest - b > 0 else [a, b]
    if ntiles <= 2:
        pieces = [1] * ntiles
    else:
        pieces = []
        rem = ntiles
        while rem > 2:
            pieces.append(2)
            rem -= 2
        while rem:
            pieces.append(1)
            rem -= 1
    return chunks, pieces


def _build_program(ntiles, tail_idx, warmup):
    """One SPMD core program.

    Inputs (per core):
      blob [128, 512 + ntiles*256] fp16 —
        cols [0,512):   wk[p, kt*256+o] = W[e, kt*128+p, o]
        cols [512,..):  xk[p, t*256+kt*128+j] = x_tok(t*128+j)[kt*128+p]
    Output:
      yk [ntiles*128, 256] fp16 — yk[s] = y of the s-th token in this
      core's (expert-sorted) token list; rows past the valid count are 0.
    """
    chunks, pieces = _plan(ntiles)
    assert sum(chunks) == ntiles and sum(pieces) == ntiles
    dt16 = mybir.dt.float16
    dt32 = mybir.dt.float32

    nc = bass.Bass(
        "TRN2",
        target_bir_lowering=False,
        debug=False,
        num_devices=NCORES,
        num_swdge_queues=min(4, len(pieces)),
    )
    XCOLS = 512 + ntiles * 256
    blob = nc.dram_tensor("blob", [128, XCOLS], dt16, kind="ExternalInput").ap()
    yk = nc.dram_tensor("yk", [ntiles * 128 + 128, 256], dt16, kind="ExternalOutput").ap()

    with _SplitDrainTileContext(nc) as tc:
        with ExitStack() as ctx:
            inpool = ctx.enter_context(tc.tile_pool(name="in", bufs=1))
            ppool = ctx.enter_context(tc.tile_pool(name="ps", bufs=1, space="PSUM"))
            ypool = ctx.enter_context(tc.tile_pool(name="y", bufs=1))
            mpool = ctx.enter_context(tc.tile_pool(name="misc", bufs=1))

            # --- misc tiles: scatter indices + warmup scratch
            idx_tiles = []
            base = 0
            for pi, np_ in enumerate(pieces):
                n_idx = np_ * 128 if pi < len(pieces) - 1 else tail_idx
                it = mpool.tile([128, n_idx // 16], mybir.dt.int16, tag=f"idx{pi}", name=f"idx{pi}")
                idx_tiles.append((it, n_idx, base))
                base += np_ * 128

            scratch = None
            psd = None
            if warmup:
                scratch = mpool.tile([128, 128], dt16, tag="scr", name="scr")
                psd = ppool.tile([128, 128], dt32, tag="psd", name="psd")

            # --- input DMA chunks (SP engine; HWDGE). Chunk 0 carries W.
            in_tiles = []
            col = 0
            t0 = 0
            for ci, nt in enumerate(chunks):
                w = (512 if ci == 0 else 0) + nt * 256
                tl = inpool.tile([128, w], dt16, tag=f"in{ci}", name=f"in{ci}")
                nc.sync.dma_start(out=tl[:], in_=blob[:, col : col + w])
                in_tiles.append((tl, t0, nt, 512 if ci == 0 else 0))
                col += w
                t0 += nt

            # --- early Pool work: iotas then scatter preps (descriptors only)
            for pi, (it, n_idx, pbase) in enumerate(idx_tiles):
                nc.gpsimd.iota(
                    it[:], pattern=[[16, n_idx // 16]], base=pbase, channel_multiplier=1
                )
            nc.gpsimd.load_library(library_config.mlp)
            ysb = []
            for pi, np_ in enumerate(pieces):
                ysb.append(ypool.tile([128, np_, 256], dt16, tag=f"y{pi}", name=f"ysb{pi}"))
            dma_sems = []
            for pi, np_ in enumerate(pieces):
                it, n_idx, pbase = idx_tiles[pi]
                sem = nc.alloc_semaphore(f"scat{pi}")
                dma_sems.append(sem)
                nc.gpsimd.dma_scatter_add(
                    yk,
                    ysb[pi][:],
                    it[:],
                    n_idx,
                    n_idx,
                    256,
                    prepare_only=True,
                    sem=sem,
                    queue_num=pi,
                )

            # --- PE warmup: keep the tensor engine continuously busy through
            # its p-state ramp while input DMAs are in flight.
            if warmup:
                nc.vector.memset(scratch[:], 0.0)
                for _ in range(warmup):
                    nc.tensor.matmul(
                        out=psd[:],
                        lhsT=scratch[:],
                        rhs=scratch[:],
                        start=True,
                        stop=True,
                    )

            # --- real matmuls: per 128-token tile, kt0 (start) + kt1 (stop)
            def lhsT(t, kt):
                for tl, t0, nt, off in in_tiles:
                    if t0 <= t < t0 + nt:
                        lo = off + (t - t0) * 256 + kt * 128
                        return tl[:, lo : lo + 128]
                raise AssertionError(t)

            w_tile = in_tiles[0][0]
            ps = []
            for t in range(ntiles):
                pt = ppool.tile([128, 256], dt32, tag=f"p{t}", name=f"ps{t}")
                ps.append(pt)
            for t in range(ntiles):
                for kt in range(2):
                    nc.tensor.matmul(
                        out=ps[t][:],
                        lhsT=lhsT(t, kt),
                        rhs=w_tile[:, kt * 256 : (kt + 1) * 256],
                        start=(kt == 0),
                        stop=(kt == 1),
                    )

            # --- PSUM -> SBUF fp16 copies (alternate DVE/ACT; split the last
            # tile across both engines to shorten the tail), then triggers.
            tile_piece = []
            for pi, np_ in enumerate(pieces):
                tile_piece += [(pi, li) for li in range(np_)]
            trig_done = set()

            def maybe_trigger(pi):
                # trigger piece pi once all its tiles are copied
                if pi in trig_done:
                    return
                trig_done.add(pi)
                nc.gpsimd.trigger_dma(count=None, queue_num=pi)

            copied = [0] * len(pieces)
            for t in range(ntiles):
                pi, li = tile_piece[t]
                if t == ntiles - 1:
                    nc.vector.tensor_copy(ysb[pi][:, li, 0:128], ps[t][:, 0:128])
                    nc.scalar.copy(ysb[pi][:, li, 128:256], ps[t][:, 128:256])
                elif t % 2 == 0:
                    nc.vector.tensor_copy(ysb[pi][:, li, :], ps[t][:])
                else:
                    nc.scalar.copy(ysb[pi][:, li, :], ps[t][:])
                copied[pi] += 1
                if copied[pi] == pieces[pi]:
                    maybe_trigger(pi)

    return nc


_cache: dict = {}


def _get_program(ntiles, tail_idx, warmup):
    key = (ntiles, tail_idx, warmup)
    if key not in _cache:
        _cache[key] = _build_program(ntiles, tail_idx, warmup)
    return _cache[key]


WARMUP = 24


def _route(x, Wp, bp):
    """Host router with exactly the reference fp32 semantics (incl. ties)."""
    logits = x @ Wp.T + bp
    g = 1.0 / (1.0 + np.exp(-logits, dtype=np.float32))
    onehot = g == g.max(axis=1, keepdims=True)  # [B, E] bool, >=1 True per row
    tok_of_pair, exp_of_pair = np.nonzero(onehot)
    order = np.argsort(exp_of_pair, kind="stable")
    toks_by_e = tok_of_pair[order]
    n_e = np.bincount(exp_of_pair, minlength=E)
    return toks_by_e, n_e


def _pack_inputs(x, W, toks_by_e, n_e):
    C = max(1, int(n_e.max()))
    ntiles = -(-C // 128)
    S = ntiles * 128

    x16 = x.astype(np.float16)
    in_maps = []
    tok_lists = []
    off = 0
    for c in range(NCORES):
        toks = toks_by_e[off : off + n_e[c]]
        off += n_e[c]
        tok_lists.append(toks)
        xg = np.zeros((S, 256), dtype=np.float16)
        xg[: len(toks)] = x16[toks]
        # [S,256] -> [t, j, kt, p] -> [p, t, kt, j] -> [128, ntiles*256]
        xpart = (
            xg.reshape(ntiles, 128, 2, 128)
            .transpose(3, 0, 2, 1)
            .reshape(128, ntiles * 256)
        )
        wpart = (
            W[c].astype(np.float16).reshape(2, 128, 256).transpose(1, 0, 2).reshape(128, 512)
        )
        blob = np.ascontiguousarray(np.concatenate([wpart, xpart], axis=1))
        in_maps.append({"blob": blob})
    return in_maps, tok_lists, ntiles


def kernel(x, W, Wp, bp):
    x = np.ascontiguousarray(np.asarray(x, dtype=np.float32))
    W = np.ascontiguousarray(np.asarray(W, dtype=np.float32))
    Wp = np.ascontiguousarray(np.asarray(Wp, dtype=np.float32))
    bp = np.ascontiguousarray(np.asarray(bp, dtype=np.float32))
    B = x.shape[0]

    toks_by_e, n_e = _route(x, Wp, bp)
    in_maps, tok_lists, ntiles = _pack_inputs(x, W, toks_by_e, n_e)

    _, pieces = _plan(ntiles)
    last_base = (ntiles - sum(pieces[-1:])) * 128
    tail_valid = int(n_e.max()) - last_base
    tail_idx = max(16, -(-tail_valid // 16) * 16)

    nc = _get_program(ntiles, tail_idx, WARMUP)
    res = run_bass_kernel_spmd(nc, in_maps, list(range(NCORES)))

    y = np.zeros((B, O), dtype=np.float32)
    for c in range(NCORES):
        yc = res.results[c]["yk"].astype(np.float32)  # [ntiles*128, 256]
        toks = tok_lists[c]
        np.add.at(y, toks, yc[: len(toks)])
    return y


# revision 24
# speedup vs baseline: 2.2420x; 1.5644x over previous
"""Top-1 MoE routing layer (HCE Linear) on 8 Trainium2 NeuronCores.

y[b] = x[b] @ W[argmax_e sigmoid(x @ Wp.T + bp)[b, e]]   (multi-hot on exact ties)

Strategy: EXPERT-parallel, fp16 compute with fp32 PSUM accumulation. The
router runs on host in fp32 with exactly the reference semantics; core e
receives expert e's weight and the tokens routed to it (zero-padded to a
whole number of 128-token tiles).

All data movement runs through the GPSIMD custom-DMA path: inputs are pulled
with plain dma_gather (iota indices over host-sorted blobs; the transposing
gather produces the x^T tiles the PE wants directly), and the output leaves
through one prepared dma_scatter_add fired by trigger_dma once the PSUM->SBUF
copies land. The PE computes y token-major (out = xT-tile @ W), so the
scatter can stream token rows straight to HBM; the host applies the final
(free) unscatter. Only SWDGE queue 0 is used — multi-queue and multi-trigger
splits corrupt data on this runtime.
"""

from contextlib import ExitStack

import numpy as np

import bass_rust
import concourse.bacc as bacc
import concourse.bass as bass
import concourse.tile as tile
from bass_rust import InstructionNameOrderedSet
from concourse import mybir
from concourse.bass_utils import run_bass_kernel_spmd
from concourse.vector_clock import ScopedClock

NCORES = 8
E = 8
I = 256
O = 256


class _SplitDrainTileContext(tile.TileContext):
    """TileContext legalized for a walrus build that allows at most ONE sem
    wait per instruction ("Too many sync wait commands" otherwise).

    Extra waits are hoisted onto same-engine InstNoOp carriers placed
    immediately before the owning instruction (identical semantics: the
    engine sequencer executes them in order), and the kernel-tail drain is
    split into a chain of single-wait drains.
    """

    _wait_nop_counter = 0

    def _lower_ordered_insts(self, ordered):
        for bb_name, insts in list(ordered.items()):
            out = []
            for inst in insts:
                si = getattr(inst, "sync_info", None)
                waits = list(si.on_wait) if si is not None else []
                if len(waits) > 1:
                    for w in waits[:-1]:
                        type(self)._wait_nop_counter += 1
                        nop = mybir.InstNoOp(
                            name=f"waitnop_{type(self)._wait_nop_counter}",
                            engine=inst.engine,
                            sync_info=mybir.SyncInfo(on_wait=[w], on_update=[]),
                            bass_nofuse=True,
                        )
                        out.append(nop)
                    inst.sync_info = mybir.SyncInfo(
                        on_wait=[waits[-1]], on_update=list(si.on_update)
                    )
                out.append(inst)
            ordered[bb_name] = out
        return super()._lower_ordered_insts(ordered)

    def _drain_and_barrier(self, tick_clock, wait_clock):
        drain_inst = self.nc.sync.drain()
        wait_clock.add_sem_waits(
            drain_inst.ins, ScopedClock({None: tick_clock.global_clock})
        )
        si = drain_inst.ins.sync_info
        waits = list(si.on_wait)
        if len(waits) > 1:
            # strip the drain; carry each wait on a cheap nop instead of a
            # chain of full drains (those cost ~100ns each)
            drain_inst.ins.sync_info = bass_rust.SyncInfo(
                on_wait=[], on_update=list(si.on_update)
            )
            for w in waits:
                n2 = self.nc.sync.nop(nofuse=True)
                n2.ins.sync_info = bass_rust.SyncInfo(on_wait=[w], on_update=[])
        self.nc.all_engine_barrier(sem_only=True)
        assert self.sems is not None
        popped = self.nc._tile_sem_poison_stack.pop()
        assert popped is self._sem_poison
        self.nc.clear_and_free_semaphores(list(self.sems.allocated().values()))


def _params(C):
    """ntiles and the scatter index count for a given max token count."""
    ntiles = max(1, -(-C // 128))
    tail_idx = max(16, -(-C // 16) * 16)
    return ntiles, tail_idx


def _build_program(ntiles, tail_idx):
    """One SPMD core program.

    Inputs (per core):
      wb [384, 256]          fp16 — rows 0..255 = W[e] (k-major); pad rows 0
      xb [ntiles*128+128, 256] fp16 — row s = x of the s-th token in this
                              core's expert-sorted list; pad rows 0
    Output:
      yk [ntiles*128+128, 256] fp16 — row s = y of token s; pad rows 0.
    """
    S = ntiles * 128
    dt16 = mybir.dt.float16
    dt32 = mybir.dt.float32

    nc = bacc.Bacc(
        "TRN2",
        target_bir_lowering=False,
        debug=False,
        num_devices=NCORES,
    )
    wb = nc.dram_tensor("wb", [384, 256], dt16, kind="ExternalInput").ap()
    xb = nc.dram_tensor("xb", [S + 128, 256], dt16, kind="ExternalInput").ap()
    yk = nc.dram_tensor("yk", [S + 128, 256], dt16, kind="ExternalOutput").ap()

    # x gathers: first 2 tiles as one chunk (gates the first matmuls), rest
    # as a second chunk
    c1 = min(2, ntiles)
    xchunks = [c1] + ([ntiles - c1] if ntiles > c1 else [])

    with _SplitDrainTileContext(nc) as tc:
        with ExitStack() as ctx:
            gpool = ctx.enter_context(tc.tile_pool(name="g", bufs=1))
            ppool = ctx.enter_context(tc.tile_pool(name="ps", bufs=1, space="PSUM"))
            mpool = ctx.enter_context(tc.tile_pool(name="misc", bufs=1))

            pool_chain = []

            # --- iota index tiles (standard library, before any mlp op)
            idxw = mpool.tile([128, 16], mybir.dt.int16, tag="idxw", name="idxw")
            idxx = mpool.tile([128, S // 16], mybir.dt.int16, tag="idxx", name="idxx")
            idxs = mpool.tile(
                [128, tail_idx // 16], mybir.dt.int16, tag="idxs", name="idxs"
            )
            pool_chain.append(
                nc.gpsimd.iota(idxw[:], pattern=[[16, 16]], base=0, channel_multiplier=1)
            )
            pool_chain.append(
                nc.gpsimd.iota(
                    idxx[:], pattern=[[16, S // 16]], base=0, channel_multiplier=1
                )
            )
            pool_chain.append(
                nc.gpsimd.iota(
                    idxs[:], pattern=[[16, tail_idx // 16]], base=0, channel_multiplier=1
                )
            )

            # --- input gathers (Pool): W kt0 half, x tiles 0..1, W kt1 half,
            # remaining x tiles. The transposing x gather lands x^T directly.
            gw = gpool.tile([128, 2, 256], dt16, tag="gw", name="gw")
            gxs = []
            for ci, nt in enumerate(xchunks):
                gxs.append(
                    gpool.tile([128, 2, nt * 128], dt16, tag=f"gx{ci}", name=f"gx{ci}")
                )

            def gather_w(kt):
                g = nc.gpsimd.dma_gather(
                    gw[:, kt : kt + 1, :],
                    wb,
                    idxw[:, kt * 8 : (kt + 1) * 8],
                    128,
                    128,
                    256,
                )
                pool_chain.append(g)

            def gather_x(ci):
                base = sum(xchunks[:ci])
                nt = xchunks[ci]
                g = nc.gpsimd.dma_gather(
                    gxs[ci][:],
                    xb,
                    idxx[:, base * 8 : (base + nt) * 8],
                    nt * 128,
                    nt * 128,
                    256,
                    transpose=True,
                )
                pool_chain.append(g)

            gather_w(0)
            gather_x(0)
            gather_w(1)
            if len(xchunks) > 1:
                gather_x(1)

            # --- prepared output scatter (queue 0, one piece)
            ysb = nc.alloc_sbuf_tensor("ysb", [128, ntiles, 256], dt16)
            dma_sem = nc.alloc_semaphore("scat")
            prep = nc.gpsimd.dma_scatter_add(
                yk,
                ysb.ap(),
                idxs[:],
                tail_idx,
                tail_idx,
                256,
                prepare_only=True,
                sem=dma_sem,
                queue_num=0,
            )
            pool_chain.append(prep)
            nc.gpsimd._pending_untriggered_insts[0] = []

            # --- matmuls: per 128-token tile, kt0 (start) + kt1 (stop);
            # out[token, o] so the scatter can stream token rows.
            def lhsT(t, kt):
                off = 0
                for ci, nt in enumerate(xchunks):
                    if t < off + nt:
                        return gxs[ci][:, kt, (t - off) * 128 : (t - off + 1) * 128]
                    off += nt
                raise AssertionError(t)

            ps = []
            for t in range(ntiles):
                ps.append(ppool.tile([128, 256], dt32, tag=f"p{t}", name=f"ps{t}"))
            for t in range(ntiles):
                for kt in range(2):
                    nc.tensor.matmul(
                        out=ps[t][:],
                        lhsT=lhsT(t, kt),
                        rhs=gw[:, kt, :],
                        start=(kt == 0),
                        stop=(kt == 1),
                    )

            # --- PSUM -> SBUF fp16 copies: alternate DVE/ACT; the last tile
            # splits across both engines to shorten the tail.
            for t in range(ntiles):
                dst = ysb.ap()
                if t == ntiles - 1 and ntiles > 1:
                    nc.vector.tensor_copy(dst[:, t, 0:128], ps[t][:, 0:128])
                    nc.scalar.copy(dst[:, t, 128:256], ps[t][:, 128:256])
                elif t % 2 == 0:
                    nc.vector.tensor_copy(dst[:, t, :], ps[t][:])
                else:
                    nc.scalar.copy(dst[:, t, :], ps[t][:])

            # --- fire the scatter once the copies land (framework attaches
            # the ysb-writer deps to the trigger via the pending-list link)
            nc.gpsimd._pending_untriggered_insts[0] = [prep]
            trig = nc.gpsimd.trigger_dma(count=1)
            pool_chain.append(trig)
            pool_chain.append(nc.gpsimd.wait_ge(dma_sem, 16))

            # pin the Pool stream order — the scheduler would otherwise float
            # the dep-less waits/triggers ahead of the gathers and prep.
            for a, b in zip(pool_chain, pool_chain[1:]):
                deps = InstructionNameOrderedSet()
                deps.add(a.ins.name)
                b.ins.add_nosync_dependencies_from(deps)

    return nc


_cache: dict = {}


def _get_program(ntiles, tail_idx):
    key = (ntiles, tail_idx)
    if key not in _cache:
        nc = _build_program(ntiles, tail_idx)
        if not nc.is_finalized():
            nc.finalize()
        _cache[key] = nc
    return _cache[key]


def _route(x, Wp, bp):
    """Host router with exactly the reference fp32 semantics (incl. ties)."""
    logits = x @ Wp.T + bp
    g = 1.0 / (1.0 + np.exp(-logits, dtype=np.float32))
    onehot = g == g.max(axis=1, keepdims=True)  # [B, E] bool, >=1 True per row
    tok_of_pair, exp_of_pair = np.nonzero(onehot)
    order = np.argsort(exp_of_pair, kind="stable")
    toks_by_e = tok_of_pair[order]
    n_e = np.bincount(exp_of_pair, minlength=E)
    return toks_by_e, n_e


def _pack_inputs(x, W, toks_by_e, n_e):
    C = max(1, int(n_e.max()))
    ntiles, _ = _params(C)
    S = ntiles * 128

    x16 = x.astype(np.float16)
    in_maps = []
    tok_lists = []
    off = 0
    for c in range(NCORES):
        toks = toks_by_e[off : off + n_e[c]]
        off += n_e[c]
        tok_lists.append(toks)
        xb = np.zeros((S + 128, 256), dtype=np.float16)
        xb[: len(toks)] = x16[toks]
        wbuf = np.zeros((384, 256), dtype=np.float16)
        wbuf[:256] = W[c].astype(np.float16)
        in_maps.append({"wb": wbuf, "xb": xb})
    return in_maps, tok_lists, ntiles


def kernel(x, W, Wp, bp):
    x = np.ascontiguousarray(np.asarray(x, dtype=np.float32))
    W = np.ascontiguousarray(np.asarray(W, dtype=np.float32))
    Wp = np.ascontiguousarray(np.asarray(Wp, dtype=np.float32))
    bp = np.ascontiguousarray(np.asarray(bp, dtype=np.float32))
    B = x.shape[0]

    toks_by_e, n_e = _route(x, Wp, bp)
    in_maps, tok_lists, ntiles = _pack_inputs(x, W, toks_by_e, n_e)
    _, tail_idx = _params(max(1, int(n_e.max())))

    nc = _get_program(ntiles, tail_idx)
    res = run_bass_kernel_spmd(nc, in_maps, list(range(NCORES)))

    y = np.zeros((B, O), dtype=np.float32)
    for c in range(NCORES):
        yc = res.results[c]["yk"].astype(np.float32)
        toks = tok_lists[c]
        np.add.at(y, toks, yc[: len(toks)])
    return y
